# revision 17
# baseline (speedup 1.0000x reference)
"""DGI (Deep Graph Infomax) kernel for 8 Trainium2 NeuronCores.

Strategy (row-wise graph partitioning, per the sharding hint):
  - nodes split 12500/core (padded to 12544 = 98*128 rows); each core owns the
    incoming edges of its node block.
  - phase 1: each core computes its shard of xtheta = x @ W^T + b for both
    graphs directly in node-major layout (x-tile is the stationary matmul
    operand; bias added via a K=1 accumulating matmul), rows stored as
    [node, pos_h(64) | neg_h(64)] bf16 (256B), AllGather -> full
    100352-row table in HBM.
  - phase 2: per-edge gather of the 256B source rows via gpsimd dma_gather
    (int16 indices => table is processed as 4 buckets of 25088 rows, one
    SWDGE queue per bucket). Segment-sum via matmul: for each group of 128
    destination rows, the gathered 128-edge blocks (lhsT, [e, h2]) are
    multiplied by host-precomputed one-hot value matrices
    M[e, r] = v_e * (row_e == r) streamed from HBM (sequential DMA),
    accumulating agg^T[h2, r] in PSUM. PReLU + mean-readout row-sum fused
    into a single scalar-engine activation(Prelu, accum_out=...).
  - phase 3: mean over nodes via AllReduce, sigmoid, z = W_bil @ s, then
    scores^T[{pos,neg}, node] = z2^T @ HT in N=448 strips (z2 stationary);
    the bilinear bias is added on the host during unsharding.
"""

import sys

try:
    import concourse.bacc as bacc
except ImportError:  # pragma: no cover
    sys.path.insert(0, "/opt/trn_rl_repo")
    import concourse.bacc as bacc

import numpy as np
import ml_dtypes

import concourse.bass as bass
import concourse.mybir as mybir
import concourse.tile as tile
from concourse.library_config import mlp
from concourse.bass_utils import run_bass_kernel_spmd

P = 128
BF16 = mybir.dt.bfloat16
F32 = mybir.dt.float32
I16 = mybir.dt.int16

_NC_CACHE = {}


# --------------------------------------------------------------------------
# host-side planning
# --------------------------------------------------------------------------

class Plan:
    pass


def make_plan(n_nodes, ncores, edge_rows, edge_cols, edge_vals, g_chunk=7):
    """Static (shared-across-cores) schedule + per-core edge data arrays."""
    pl = Plan()
    local_n = n_nodes // ncores
    assert local_n * ncores == n_nodes
    local_pad = ((local_n + P - 1) // P) * P
    groups = local_pad // P
    n_buckets = 4
    assert ncores % n_buckets == 0
    ranks_per_bucket = ncores // n_buckets
    bucket_rows = ranks_per_bucket * local_pad
    assert bucket_rows <= 32767, bucket_rows

    pl.ncores, pl.local_n, pl.local_pad = ncores, local_n, local_pad
    pl.groups, pl.n_buckets, pl.bucket_rows = groups, n_buckets, bucket_rows
    pl.trows = ncores * local_pad

    r = np.asarray(edge_rows).astype(np.int64)
    c = np.asarray(edge_cols).astype(np.int64)
    v = np.asarray(edge_vals).astype(np.float32)

    core = r // local_n
    lr = r % local_n
    g = lr // P
    rloc = lr % P
    crank = c // local_n
    cloc = c % local_n
    q = crank // (2 if ranks_per_bucket == 2 else ranks_per_bucket)
    q = crank // ranks_per_bucket
    idx16 = (crank % ranks_per_bucket) * local_pad + cloc

    # per (core, g, q) segment counts
    key = (core * groups + g) * n_buckets + q
    counts = np.bincount(key, minlength=ncores * groups * n_buckets).reshape(
        ncores, groups, n_buckets
    )
    nblk = np.ceil(counts.max(axis=0) / P).astype(np.int64)  # [groups, n_buckets]
    # every group needs at least one block so its PSUM tile gets written
    empty_g = nblk.sum(axis=1) == 0
    nblk[empty_g, 0] = 1
    pl.nblk = nblk

    # chunk structure
    chunk_ids = [list(range(k, min(k + g_chunk, groups))) for k in range(0, groups, g_chunk)]
    pl.chunks = []
    jglobal = 0
    idx_off = 0  # in int16 free-columns of the [128, *] gidx tensor
    for chunk in chunk_ids:
        spec = Plan()
        spec.groups = chunk
        spec.idx_off = idx_off
        spec.nq = []
        spec.q_off = []   # offset inside this chunk's idx tile, int16 cols
        spec.blocks = {gg: [] for gg in chunk}
        qo = 0
        for qq in range(n_buckets):
            nq = int(sum(nblk[gg][qq] for gg in chunk))
            spec.nq.append(nq)
            spec.q_off.append(qo)
            pos = 0
            for gg in chunk:
                for _ in range(int(nblk[gg][qq])):
                    spec.blocks[gg].append((qq, pos, jglobal))
                    pos += 1
                    jglobal += 1
            qo += nq * 8  # nq*128 idxs -> /16 cols
        spec.idx_len = qo
        idx_off += qo
        js = [j for bl in spec.blocks.values() for (_, _, j) in bl]
        spec.j0, spec.j1 = min(js), max(js) + 1
        pl.chunks.append(spec)
    pl.b_total = jglobal
    pl.gidx_cols = idx_off

    # ---- per-core data arrays ----
    # sort edges by (core, g, q, idx16) so segments are contiguous
    order = np.lexsort((idx16, q, g, core))
    so_core, so_g, so_q = core[order], g[order], q[order]
    so_idx, so_rloc, so_v = idx16[order], rloc[order], v[order]
    seg_key = ((so_core * groups + so_g) * n_buckets + so_q)
    seg_counts = np.bincount(seg_key, minlength=ncores * groups * n_buckets)
    seg_starts = np.concatenate([[0], np.cumsum(seg_counts)])

    pl.gidx = []
    pl.mb = []
    for cc in range(ncores):
        all_idx = np.zeros(pl.b_total * P, np.int64)
        all_rloc = np.zeros(pl.b_total * P, np.float32)
        all_v = np.zeros(pl.b_total * P, np.float32)
        wpos = 0
        for spec in pl.chunks:
            for qq in range(n_buckets):
                for gg in spec.groups:
                    sk = (cc * groups + gg) * n_buckets + qq
                    s0, s1 = seg_starts[sk], seg_starts[sk + 1]
                    cnt = s1 - s0
                    slots = int(nblk[gg][qq]) * P
                    assert cnt <= slots
                    all_idx[wpos:wpos + cnt] = so_idx[s0:s1]
                    all_rloc[wpos:wpos + cnt] = so_rloc[s0:s1]
                    all_v[wpos:wpos + cnt] = so_v[s0:s1]
                    wpos += slots
        assert wpos == pl.b_total * P
        # wrap idx per gather call
        wrapped = []
        for spec in pl.chunks:
            base = 0
            for qq in range(n_buckets):
                nq = spec.nq[qq]
                if nq == 0:
                    continue
        # call boundaries: iterate chunks/q again tracking global edge pos
        pos = 0
        for spec in pl.chunks:
            for qq in range(n_buckets):
                nidx = spec.nq[qq] * P
                if nidx == 0:
                    continue
                sl = all_idx[pos:pos + nidx]
                w = sl.reshape(nidx // 16, 16).T.astype(np.int16)  # [16, nidx/16]
                wrapped.append(np.tile(w, (8, 1)))
                pos += nidx
        assert pos == pl.b_total * P
        gidx = np.concatenate(wrapped, axis=1)
        assert gidx.shape == (P, pl.gidx_cols)
        pl.gidx.append(np.ascontiguousarray(gidx))
        # precomputed one-hot value matrices: mb[p, j*128+d] = v at d==rloc
        arr = np.zeros((pl.b_total, P, P), ml_dtypes.bfloat16)
        jj = np.repeat(np.arange(pl.b_total), P)
        pp = np.tile(np.arange(P), pl.b_total)
        dd = all_rloc.astype(np.int64)
        vvb = all_v.astype(ml_dtypes.bfloat16)
        arr[jj, pp, dd] = vvb
        pl.mb.append(np.ascontiguousarray(
            arr.transpose(1, 0, 2).reshape(P, pl.b_total * P)))
    return pl


# --------------------------------------------------------------------------
# device kernel build
# --------------------------------------------------------------------------

class _EarlyStop(Exception):
    pass


def _early_out(nc, tc, scores_d, local_pad):
    with tc.tile_pool(name="eo", bufs=1) as eo:
        scr = eo.tile([2, local_pad], F32)
        nc.vector.memset(scr[:], 0.0)
        nc.sync.dma_start(scores_d[:], scr[:])


def build_nc(pl, stop_after=None, timing_variant=False, repeat=1):
    ncores, local_pad, groups = pl.ncores, pl.local_pad, pl.groups
    BR, trows = pl.bucket_rows, pl.trows
    stops = {"empty": -1, "lin": 0, "ag": 1, "gatheronly": 2, "mbuild": 2.2, "mm": 2.5, "p2a": 2.8, "p2b": 2.9, "phase2": 3}
    level = stops.get(stop_after, 99)

    nc = bacc.Bacc("TRN2", target_bir_lowering=False, debug=False,
                   num_devices=ncores, enable_asserts=False,
                   num_swdge_queues=4)

    # inputs
    xTp = nc.dram_tensor("xTp", [P, local_pad], F32, kind="ExternalInput")
    xTn = nc.dram_tensor("xTn", [P, local_pad], F32, kind="ExternalInput")
    w2t = nc.dram_tensor("w2t", [P, 64], F32, kind="ExternalInput")
    bg = nc.dram_tensor("bg", [1, 64], F32, kind="ExternalInput")
    wbt = nc.dram_tensor("wbt", [64, 64], F32, kind="ExternalInput")
    acol = nc.dram_tensor("acol", [P, 1], F32, kind="ExternalInput")
    bbcol = nc.dram_tensor("bbcol", [P, 1], F32, kind="ExternalInput")
    gidx_d = nc.dram_tensor("gidx", [P, pl.gidx_cols], I16, kind="ExternalInput")
    mb_d = nc.dram_tensor("mbv", [P, pl.b_total * P], BF16, kind="ExternalInput")

    scores_d = nc.dram_tensor("scores", [2, local_pad], F32, kind="ExternalOutput")

    # internal DRAM
    xt_c = nc.dram_tensor("xt_c", [local_pad, P], BF16)
    if timing_variant:
        # collective-free build for chained timing runs: the gather table is
        # supplied directly as an input, AllReduce becomes a local copy
        xt_all = nc.dram_tensor("xt_fake", [trows, P], BF16,
                                kind="ExternalInput")
        ar_in = nc.dram_tensor("ar_in", [64, 1], F32)
        ar_out = nc.dram_tensor("ar_out", [64, 1], F32)
    else:
        xt_all = nc.dram_tensor("xt_all", [trows, P], BF16, addr_space="Shared")
        ar_in = nc.dram_tensor("ar_in", [64, 1], F32)
        ar_out = nc.dram_tensor("ar_out", [64, 1], F32, addr_space="Shared")

    rg = [list(range(ncores))]
    inv_n = 1.0 / float(pl.local_n * ncores)

    with tile.TileContext(nc) as tc:
        nc.gpsimd.load_library(mlp)
        with (
            tc.tile_pool(name="const", bufs=1) as cpool,
            tc.tile_pool(name="big", bufs=1) as bigpool,
        ):
            w2t_sb = cpool.tile([P, 64], F32)
            nc.sync.dma_start(w2t_sb[:], w2t[:])
            bg_sb = cpool.tile([1, 64], F32)
            nc.sync.dma_start(bg_sb[:], bg[:])
            wbt_sb = cpool.tile([64, 64], F32)
            nc.sync.dma_start(wbt_sb[:], wbt[:])
            a_sb = cpool.tile([P, 1], F32)
            nc.sync.dma_start(a_sb[:], acol[:])
            bb_sb = cpool.tile([P, 1], F32)
            nc.sync.dma_start(bb_sb[:], bbcol[:])
            HT = bigpool.tile([P, local_pad], F32)
            acc = bigpool.tile([P, groups], F32)

            for _rep in range(repeat):
                _one_pass(nc, tc, pl, level, timing_variant, rg, inv_n,
                          xTp, xTn, xt_c, xt_all, gidx_d, mb_d, scores_d,
                          ar_in, ar_out,
                          w2t_sb, bg_sb, wbt_sb, a_sb, bb_sb, HT, acc)
            if level < 4:
                _early_out(nc, tc, scores_d, local_pad)

    nc.compile()
    return nc


def _one_pass(nc, tc, pl, level, timing_variant, rg, inv_n,
              xTp, xTn, xt_c, xt_all, gidx_d, mb_d, scores_d, ar_in, ar_out,
              w2t_sb, bg_sb, wbt_sb, a_sb, bb_sb, HT, acc):
    ncores, local_pad, groups = pl.ncores, pl.local_pad, pl.groups
    BR = pl.bucket_rows
    if level >= 0:
        if True:
            # ---------------- phase 1: linear (node-major, no transpose) ---
            with (
                tc.tile_pool(name="lin", bufs=4) as lpool,
                tc.tile_pool(name="lones", bufs=1) as lones,
                tc.tile_pool(name="lpsum", bufs=4, space="PSUM") as lpsum,
            ):
                ones_r = lones.tile([1, P], F32)
                nc.vector.memset(ones_r[:], 1.0)
                for t in range(groups):
                    sl = slice(t * P, (t + 1) * P)
                    xp = lpool.tile([P, P], F32, tag="xp")
                    nc.sync.dma_start(xp[:], xTp[:, sl])
                    xn = lpool.tile([P, P], F32, tag="xn")
                    nc.sync.dma_start(xn[:], xTn[:, sl])
                    pt = lpsum.tile([P, P], F32, tag="pt")
                    nc.tensor.matmul(pt[:, 0:64], lhsT=xp[:], rhs=w2t_sb[:, 0:64],
                                     start=True, stop=False)
                    nc.tensor.matmul(pt[:, 0:64], lhsT=ones_r[:], rhs=bg_sb[0:1, :],
                                     start=False, stop=True)
                    nc.tensor.matmul(pt[:, 64:128], lhsT=xn[:], rhs=w2t_sb[:, 0:64],
                                     start=True, stop=False)
                    nc.tensor.matmul(pt[:, 64:128], lhsT=ones_r[:], rhs=bg_sb[0:1, :],
                                     start=False, stop=True)
                    xrow = lpool.tile([P, 128], BF16, tag="xrow")
                    nc.vector.tensor_copy(xrow[:], pt[:])
                    nc.sync.dma_start(xt_c[sl, :], xrow[:])

            # ---------------- all-gather xtheta ----------------------------
            if level >= 1 and not timing_variant:
                nc.gpsimd.collective_compute(
                    "AllGather", mybir.AluOpType.bypass, replica_groups=rg,
                    ins=[xt_c.ap().opt()], outs=[xt_all.ap().opt()],
                )

            # ---------------- phase 2: gather + segment-sum matmul ---------
            if level >= 2:
                with (
                    tc.tile_pool(name="gath", bufs=2) as gpool,
                    tc.tile_pool(name="idxp", bufs=2) as ipool,
                    tc.tile_pool(name="mb", bufs=2) as mpool,
                    tc.tile_pool(name="gpsum", bufs=4, space="PSUM") as gpsum,
                ):
                    for spec in pl.chunks:
                        idxt = ipool.tile([P, spec.idx_len], I16, tag="idx")
                        nc.sync.dma_start(
                            idxt[:],
                            gidx_d[:, spec.idx_off:spec.idx_off + spec.idx_len])
                        nmb = spec.j1 - spec.j0
                        mbt = mpool.tile([P, nmb * P], BF16, tag="mbs")
                        nc.sync.dma_start(
                            mbt[:], mb_d[:, spec.j0 * P:spec.j1 * P])
                        gds = {}
                        for qq in range(pl.n_buckets):
                            nq = spec.nq[qq]
                            if nq == 0:
                                continue
                            gd = gpool.tile([P, nq, P], BF16, tag=f"gd{qq}")
                            nidx = nq * P
                            qo = spec.q_off[qq]
                            nc.gpsimd.dma_gather(
                                gd[:], xt_all[qq * BR:(qq + 1) * BR, :],
                                idxt[:, qo:qo + nidx // 16], nidx, nidx, P,
                                single_packet=(nidx <= 1024),
                                queue_num=qq,
                            )
                            gds[qq] = gd
                        if level < 2.2:
                            continue
                        for gg in spec.groups:
                            blocks = spec.blocks[gg]
                            pg = gpsum.tile([P, 128], F32, tag="pg")
                            nb = len(blocks)
                            for i, (qq, pos, j) in enumerate(blocks):
                                jo = j - spec.j0
                                nc.tensor.matmul(pg[:], lhsT=gds[qq][:, pos, :],
                                                 rhs=mbt[:, jo * P:(jo + 1) * P],
                                                 start=(i == 0),
                                                 stop=(i == nb - 1))
                            if level < 2.7:
                                nc.vector.tensor_copy(
                                    HT[:, gg * P:(gg + 1) * P], pg[:])
                                continue
                            nc.scalar.activation(
                                HT[:, gg * P:(gg + 1) * P], pg[:],
                                mybir.ActivationFunctionType.Prelu,
                                alpha=a_sb[:, 0:1],
                                accum_out=acc[:, gg:gg + 1])

            # ---------------- phase 3: readout + scores --------------------
            if level >= 4:
                with (
                    tc.tile_pool(name="ro", bufs=1) as ro,
                    tc.tile_pool(name="rpsum", bufs=1, space="PSUM") as rpsum,
                ):
                    msum = ro.tile([P, 1], F32)
                    nc.vector.reduce_sum(msum[:], acc[:],
                                         axis=mybir.AxisListType.X)
                    nc.sync.dma_start(ar_in[:], msum[0:64, :])
                    if timing_variant:
                        arb = ro.tile([64, 1], F32)
                        nc.sync.dma_start(arb[:], ar_in[:])
                        nc.sync.dma_start(ar_out[:], arb[:])
                    else:
                        nc.gpsimd.collective_compute(
                            "AllReduce", mybir.AluOpType.add, replica_groups=rg,
                            ins=[ar_in.ap().opt()], outs=[ar_out.ap().opt()],
                        )
                    ssum = ro.tile([64, 1], F32)
                    nc.sync.dma_start(ssum[:], ar_out[:])
                    sig = ro.tile([64, 1], F32)
                    nc.scalar.activation(sig[:], ssum[:],
                                         mybir.ActivationFunctionType.Sigmoid,
                                         scale=inv_n)
                    zp = rpsum.tile([64, 1], F32, tag="zp")
                    nc.tensor.matmul(zp[:], lhsT=wbt_sb[:], rhs=sig[:],
                                     start=True, stop=True)
                    z2 = ro.tile([P, 2], F32)
                    nc.vector.memset(z2[:], 0.0)
                    nc.scalar.copy(z2[0:64, 0:1], zp[:])
                    nc.scalar.copy(z2[64:128, 1:2], zp[:])
                    # scores^T[{pos,neg}, dst] = z2^T @ HT, N=512 strips
                    scr = ro.tile([2, local_pad], F32)
                    nstrip = local_pad // 448
                    for t in range(nstrip):
                        ssl = slice(t * 448, (t + 1) * 448)
                        spt = rpsum.tile([2, 448], F32, tag="sp")
                        nc.tensor.matmul(spt[:], lhsT=z2[:], rhs=HT[:, ssl],
                                         start=True, stop=True)
                        nc.vector.tensor_copy(scr[:, ssl], spt[:])
                    nc.sync.dma_start(scores_d[:], scr[:])


def _make_in_maps(pl, inputs):
    ncores = pl.ncores
    pos, neg = inputs["pos"], inputs["neg"]
    local_n, local_pad = pl.local_n, pl.local_pad
    a_val = np.float32(np.asarray(inputs["prelu_a"]).reshape(-1)[0])
    bb_val = np.float32(np.asarray(inputs["b_bil"]).reshape(-1)[0])
    w2t = np.ascontiguousarray(np.asarray(inputs["W_gcn"]).T.astype(np.float32))
    wbt = np.ascontiguousarray(np.asarray(inputs["W_bil"]).T.astype(np.float32))
    bgv = np.asarray(inputs["b_gcn"]).reshape(1, 64).astype(np.float32)

    in_maps = []
    for c in range(ncores):
        sl = slice(c * local_n, (c + 1) * local_n)
        xtp = np.zeros((P, local_pad), np.float32)
        xtp[:, :local_n] = np.asarray(pos[0, sl, :]).T
        xtn = np.zeros((P, local_pad), np.float32)
        xtn[:, :local_n] = np.asarray(neg[0, sl, :]).T
        in_maps.append({
            "xTp": xtp,
            "xTn": xtn,
            "w2t": w2t,
            "bg": bgv,
            "wbt": wbt,
            "acol": np.full((P, 1), a_val, np.float32),
            "bbcol": np.full((P, 1), bb_val, np.float32),
            "gidx": pl.gidx[c],
            "mbv": pl.mb[c],
        })
    return in_maps


def _assemble(pl, results, n_total, b_bil):
    ncores, local_n = pl.ncores, pl.local_n
    bb = np.float32(np.asarray(b_bil).reshape(-1)[0])
    logits = np.zeros((1, 2 * n_total), np.float32)
    for c in range(ncores):
        arr = results[c]["scores"]            # [2, local_pad]
        logits[0, c * local_n:(c + 1) * local_n] = arr[0, :local_n] + bb
        logits[0, n_total + c * local_n:n_total + (c + 1) * local_n] = \
            arr[1, :local_n] + bb
    return logits


def _run(pos, neg, edge_rows, edge_cols, edge_vals,
         W_gcn, b_gcn, prelu_a, W_bil, b_bil, ncores=8, **run_kwargs):
    n_nodes = pos.shape[1]
    f_dim = pos.shape[2]
    assert f_dim == P

    pl = make_plan(n_nodes, ncores, edge_rows, edge_cols, edge_vals)

    key = (n_nodes, ncores, pl.b_total, pl.gidx_cols,
           tuple(pl.nblk.reshape(-1).tolist()))
    if key in _NC_CACHE:
        nc = _NC_CACHE[key]
    else:
        nc = build_nc(pl)
        _NC_CACHE.clear()
        _NC_CACHE[key] = nc

    in_maps = _make_in_maps(pl, {
        "pos": pos, "neg": neg, "W_gcn": W_gcn, "b_gcn": b_gcn,
        "prelu_a": prelu_a, "W_bil": W_bil, "b_bil": b_bil,
    })

    res = run_bass_kernel_spmd(nc, in_maps, core_ids=list(range(ncores)),
                               **run_kwargs)

    logits = _assemble(pl, res.results, n_nodes, b_bil)
    return logits, res


def kernel(pos, neg, edge_rows, edge_cols, edge_vals,
           W_gcn, b_gcn, prelu_a, W_bil, b_bil):
    logits, _ = _run(pos, neg, edge_rows, edge_cols, edge_vals,
                     W_gcn, b_gcn, prelu_a, W_bil, b_bil)
    return logits

